# revision 4
# baseline (speedup 1.0000x reference)
"""GCN layer (SpMM segment-sum + dense projection + relu) on 8 TRN2 NeuronCores.

    AH[i] = sum_{e: src[e]==i} val[e] * H[dst[e]];  out = relu(AH @ W + b)

Strategy (src-sharded, one SPMD program on 8 cores):
  - Each core owns 12500 contiguous src rows of the output. The host buckets
    edges by owner core, sorts by src, and packs per-src segments into
    "windows" of <=128 srcs whose edges are split by dst-chunk (4 chunks of
    25000 H rows, int16-indexable) with <=512 edges per chunk -> each window
    is exactly 16 batches of 128 edge slots (4 per chunk, sentinel-padded).
  - Per gather block (4 windows) the core issues one dma_gather per chunk
    (2048 random 512B rows of H from HBM -> SBUF, one descriptor per row).
  - Per 128-edge batch, VectorE builds P[e, slot] = (iota==srclocal[e])*val[e]
    (one fused tensor_scalar), and TensorE accumulates
    AH^T[feat, slot] += G_batch^T @ P in PSUM (fp16 operands, fp32 accum).
  - Per window: AH^T -> fp32 projection matmul with W -> +b, relu -> rows of a
    per-core virtual-slot output; the host permutes virtual rows back to
    global node order (pure indexing).

All floating-point compute happens on device; the host only reorders
indices/values and permutes output rows (sharding prep).
"""
import numpy as np

import concourse.bacc as bacc
import concourse.bass as bass
import concourse.mybir as mybir
import concourse.tile as tile

N_NODES = 100000
F = 128          # feature dim == num units
N_CORES = 8
RPC = N_NODES // N_CORES   # src rows per core (12500)
NCHUNK = 4
CHUNK = 25000    # H rows per gather chunk (int16-indexable)
QB = 4           # batches per (window, chunk)
NB = NCHUNK * QB           # 16 batches (2048 edge slots) per window
BW = 4           # windows per gather block
CB = NB * BW     # 64 batch columns per block
CALL_IDX = BW * QB * 128   # 2048 idxs per dma_gather call
CDT = mybir.dt.float16     # compute dtype for the SpMM accumulation
NP_CDT = np.float16


# ----------------------------------------------------------------- host prep

def _pack_core(chunk_counts):
    """Pack srcs (in order) into windows obeying <=128 srcs and <=QB*128
    edges per chunk. chunk_counts: [RPC, 4] per-src per-chunk edge counts.
    Returns list of windows (lists of src ids)."""
    cap = QB * 128
    windows = []
    cur, used = [], np.zeros(NCHUNK, np.int64)
    for s in range(RPC):
        ck = chunk_counts[s]
        if cur and (len(cur) >= 128 or np.any(used + ck > cap)):
            windows.append(cur)
            cur, used = [], np.zeros(NCHUNK, np.int64)
        cur.append(s)
        used += ck
    if cur:
        windows.append(cur)
    return windows


def prepare(H, edge_vals, W, b, edge_src, edge_dst):
    H = np.ascontiguousarray(np.asarray(H, np.float32))
    edge_vals = np.asarray(edge_vals, np.float32)
    W = np.ascontiguousarray(np.asarray(W, np.float32))
    b = np.asarray(b, np.float32)
    edge_src = np.asarray(edge_src, np.int64)
    edge_dst = np.asarray(edge_dst, np.int64)

    per_core = []
    for c in range(N_CORES):
        sel = (edge_src // RPC) == c
        s = edge_src[sel] - c * RPC
        d = edge_dst[sel]
        v = edge_vals[sel]
        k = d // CHUNK
        # sort by (src, chunk) so each (src, chunk) run is contiguous
        order = np.lexsort((k, s))
        s, d, v, k = s[order], d[order], v[order], k[order]
        # per-src per-chunk counts
        cc = np.zeros((RPC, NCHUNK), np.int64)
        np.add.at(cc, (s, k), 1)
        windows = _pack_core(cc)
        per_core.append((s, d, v, k, cc, windows))

    W_win = max(len(pc[5]) for pc in per_core)
    W_win = -(-W_win // BW) * BW
    nblk = W_win // BW

    # per-core slot arrays in canonical order [window, chunk, q, j]
    nslots = W_win * NB * 128
    lidx = np.zeros((N_CORES, W_win, NCHUNK, QB * 128), np.int16)
    srcl = np.zeros((N_CORES, W_win, NB, 128), np.float32)
    vals = np.zeros((N_CORES, W_win, NB, 128), np.float32)
    rowmap = np.full((N_CORES, W_win * 128), -1, np.int64)

    for c in range(N_CORES):
        s, d, v, k, cc, windows = per_core[c]
        # start offset of each (src, chunk) run in the sorted edge list
        flat_counts = cc.reshape(-1)
        starts = np.zeros(RPC * NCHUNK + 1, np.int64)
        np.cumsum(flat_counts, out=starts[1:])
        for w, wsrcs in enumerate(windows):
            fill = np.zeros(NCHUNK, np.int64)
            for j, sid in enumerate(wsrcs):
                rowmap[c, w * 128 + j] = c * RPC + sid
                for ck in range(NCHUNK):
                    n = cc[sid, ck]
                    if n == 0:
                        continue
                    d0 = starts[sid * NCHUNK + ck]
                    pos = fill[ck]
                    lidx[c, w, ck, pos:pos + n] = (d[d0:d0 + n] - ck * CHUNK)
                    q0, j0 = divmod(pos, 128)
                    # slots within chunk are linear; srcl/vals are [NB, 128]
                    flat = ck * QB * 128 + pos
                    srcl.reshape(N_CORES, W_win, -1)[c, w, flat:flat + n] = j
                    vals.reshape(N_CORES, W_win, -1)[c, w, flat:flat + n] = v[d0:d0 + n]
                    fill[ck] += n

    # gather idx arrays: per (block, chunk) call of 2048 idxs.
    # call flat position i = (w_loc*QB + q)*128 + j  <-> lidx[c, blk*BW+w_loc, ck, q*128+j]
    # wrapped int16 [16, 128] replicated to [128, 128] per call.
    gidx = np.zeros((N_CORES, nblk, NCHUNK, 128, CALL_IDX // 16), np.int16)
    l5 = lidx.reshape(N_CORES, nblk, BW, NCHUNK, QB * 128)
    for c in range(N_CORES):
        for blk in range(nblk):
            for ck in range(NCHUNK):
                flat = l5[c, blk, :, ck, :].reshape(-1)  # [BW*QB*128] in (w_loc, q, j)
                wrapped = flat.reshape(CALL_IDX // 16, 16).T  # [16, 128]
                gidx[c, blk, ck] = np.tile(wrapped, (8, 1))

    # srcl/val device layout [nblk, 128, CB]: column cb = w_loc*NB + (ck*QB+q)
    def to_cols(a):
        # a: [N_CORES, W_win, NB, 128] -> [N_CORES, nblk, 128, BW*NB]
        return (a.reshape(N_CORES, nblk, BW, NB, 128)
                 .transpose(0, 1, 4, 2, 3)
                 .reshape(N_CORES, nblk, 128, CB))

    srcl = to_cols(srcl)
    vals = to_cols(vals)

    iota = np.tile(np.arange(128, dtype=NP_CDT), (128, 1))
    brep = np.tile(b, (128, 1)).astype(np.float32)

    in_maps = []
    for c in range(N_CORES):
        in_maps.append({
            "H": H,
            "gidx": gidx[c],
            "srcl": srcl[c],
            "val": vals[c],
            "iota": iota,
            "Wm": W,
            "brep": brep,
        })
    return in_maps, rowmap, W_win, nblk


# ------------------------------------------------------------- device program

def build_program(nblk, repeat=1):
    nc = bacc.Bacc("TRN2", target_bir_lowering=False, debug=False)
    H_t = nc.dram_tensor("H", [N_NODES, F], mybir.dt.float32, kind="ExternalInput")
    gidx_t = nc.dram_tensor("gidx", [nblk, NCHUNK, 128, CALL_IDX // 16],
                            mybir.dt.int16, kind="ExternalInput")
    srcl_t = nc.dram_tensor("srcl", [nblk, 128, CB], mybir.dt.float32, kind="ExternalInput")
    val_t = nc.dram_tensor("val", [nblk, 128, CB], mybir.dt.float32, kind="ExternalInput")
    iota_t = nc.dram_tensor("iota", [128, 128], CDT, kind="ExternalInput")
    Wm_t = nc.dram_tensor("Wm", [F, F], mybir.dt.float32, kind="ExternalInput")
    brep_t = nc.dram_tensor("brep", [128, F], mybir.dt.float32, kind="ExternalInput")
    out_t = nc.dram_tensor("outv", [nblk * BW * 128, F], mybir.dt.float32,
                           kind="ExternalOutput")

    with tile.TileContext(nc) as tc:
        with (
            tc.tile_pool(name="consts", bufs=1) as cpool,
            tc.tile_pool(name="gpool", bufs=2) as gpool,
            tc.tile_pool(name="gcpool", bufs=2) as gcpool,
            tc.tile_pool(name="inpool", bufs=3) as inpool,
            tc.tile_pool(name="ppool", bufs=4) as ppool,
            tc.tile_pool(name="phase2", bufs=3) as p2pool,
            tc.tile_pool(name="opool", bufs=4) as opool,
            tc.tile_pool(name="psacc", bufs=5, space="PSUM") as psacc,
            tc.tile_pool(name="psout", bufs=3, space="PSUM") as psout,
        ):
            iota_sb = cpool.tile([128, 128], CDT)
            nc.sync.dma_start(iota_sb[:], iota_t[:])
            Wm_sb = cpool.tile([F, F], mybir.dt.float32)
            nc.sync.dma_start(Wm_sb[:], Wm_t[:])
            brep_sb = cpool.tile([128, F], mybir.dt.float32)
            nc.sync.dma_start(brep_sb[:], brep_t[:])

            for blk in [b for _ in range(repeat) for b in range(nblk)]:
                srcl_sb = inpool.tile([128, CB], mybir.dt.float32, tag="srcl")
                nc.sync.dma_start(srcl_sb[:], srcl_t[blk])
                val_sb = inpool.tile([128, CB], mybir.dt.float32, tag="val")
                nc.sync.dma_start(val_sb[:], val_t[blk])

                G = gpool.tile([128, NCHUNK, BW * QB, F], mybir.dt.float32)
                for ck in range(NCHUNK):
                    gidx_sb = inpool.tile([128, CALL_IDX // 16], mybir.dt.int16,
                                          tag="gidx")
                    nc.sync.dma_start(gidx_sb[:], gidx_t[blk, ck])
                    nc.gpsimd.dma_gather(
                        out_ap=G[:, ck],
                        in_ap=H_t[ck * CHUNK:(ck + 1) * CHUNK, :],
                        idxs_ap=gidx_sb[:],
                        num_idxs=CALL_IDX,
                        num_idxs_reg=CALL_IDX,
                        elem_size=F,
                        single_packet=False,
                    )
                Gc = gcpool.tile([128, NCHUNK, BW * QB, F], CDT)
                nc.scalar.activation(
                    Gc[:].rearrange("p k c f -> p (k c f)"),
                    G[:].rearrange("p k c f -> p (k c f)"),
                    mybir.ActivationFunctionType.Copy)

                for wl in range(BW):
                    ps = psacc.tile([128, 128], mybir.dt.float32, space="PSUM")
                    for ck in range(NCHUNK):
                        for q in range(QB):
                            cb = wl * NB + ck * QB + q
                            P = ppool.tile([128, 128], CDT)
                            nc.vector.tensor_scalar(
                                out=P[:],
                                in0=iota_sb[:],
                                scalar1=srcl_sb[:, cb:cb + 1],
                                scalar2=val_sb[:, cb:cb + 1],
                                op0=mybir.AluOpType.is_equal,
                                op1=mybir.AluOpType.mult,
                            )
                            # psum[f, slot] += G_batch^T @ P
                            nc.tensor.matmul(
                                ps[:],
                                lhsT=Gc[:, ck, wl * QB + q, :],
                                rhs=P[:],
                                start=(ck == 0 and q == 0),
                                stop=(ck == NCHUNK - 1 and q == QB - 1),
                            )
                    # ---- fused phase 2 for this window ----
                    ahT_sb = p2pool.tile([128, 128], mybir.dt.float32, tag="ahT")
                    nc.vector.tensor_copy(ahT_sb[:], ps[:])
                    o_ps = psout.tile([128, F], mybir.dt.float32, space="PSUM")
                    nc.tensor.matmul(o_ps[:], lhsT=ahT_sb[:], rhs=Wm_sb[:],
                                     start=True, stop=True)
                    o_sb = opool.tile([128, F], mybir.dt.float32)
                    nc.vector.tensor_tensor(out=o_sb[:], in0=o_ps[:],
                                            in1=brep_sb[:],
                                            op=mybir.AluOpType.add)
                    nc.vector.tensor_scalar(out=o_sb[:], in0=o_sb[:],
                                            scalar1=0.0, scalar2=None,
                                            op0=mybir.AluOpType.max)
                    w = blk * BW + wl
                    nc.sync.dma_start(out_t[w * 128:(w + 1) * 128, :], o_sb[:])
    nc.compile()
    return nc


# ------------------------------------------------------------------ interface

_CACHE = {}


def _get_runner(nblk, repeat=1):
    key = (nblk, repeat)
    if key not in _CACHE:
        from runner import SpmdRunner
        _CACHE[key] = SpmdRunner(build_program(nblk, repeat), N_CORES)
    return _CACHE[key]


def kernel(H, edge_vals, W, b, edge_src, edge_dst):
    in_maps, rowmap, W_win, nblk = prepare(H, edge_vals, W, b, edge_src, edge_dst)
    runner = _get_runner(nblk)
    runner.prepare(in_maps)
    results = runner.run()
    out = np.zeros((N_NODES, F), np.float32)
    for c in range(N_CORES):
        rm = rowmap[c]
        valid = rm >= 0
        out[rm[valid]] = results[c]["outv"][valid]
    return out
